# revision 2
# baseline (speedup 1.0000x reference)
"""Multi-head attention (B=2, T=2048, D=2048, H=16) on 8 TRN2 NeuronCores.

Tensor-parallel over heads: each core computes 2 heads (dl=256 of D) of the
Q/K/V projections, its heads' attention, and a partial output projection
(columns of Wo). Host sums the 8 partial outputs (the "all-reduce").

Per-core dataflow (bf16 compute, f32 PSUM accumulation):
  QT_h = (Wq_h/sqrt(dh)) @ q^T        [dh=128, BT=4096]  (transposed layout)
  KT_h = Wk_h @ k^T                   [dh, BT]
  V    = v @ Wv_i^T                   [BT, 256]           (natural layout)
  scoresT = KT_h^T-chunk @ QT_h       [k-tile 128, q 512] per (b, h)
  attnT = exp(scoresT) * maskT        (mask multiplicative {0,1}; no max
                                       subtraction needed: |scores| < ~8)
  denom = ones^T @ attnT              (PE column-sum over k, f32 PSUM)
  O^T_h = V_h^T-chunk @ attnT scaled by 1/denom
  partial = O^T^T @ Wo_i^T            [BT, D] -> host sum over cores
"""
import sys

if "/opt/trn_rl_repo" not in sys.path:
    sys.path.insert(0, "/opt/trn_rl_repo")

import numpy as np
import ml_dtypes

import concourse.bacc as bacc
import concourse.tile as tile
import concourse.mybir as mybir
from concourse import bass_utils

BF16 = ml_dtypes.bfloat16
FP32 = mybir.dt.float32
BF = mybir.dt.bfloat16

B, T, D, H = 2, 2048, 2048, 16
DH = 128
N_CORES = 8
HL = H // N_CORES          # heads per core = 2
DL = HL * DH               # local out dim = 256
BT = B * T                 # 4096
KC = D // 128              # 16 contraction chunks
NBT = BT // 512            # 8 bt chunks
NQ = T // 512              # 4 q chunks per batch
NKT = T // 128             # 16 k tiles per batch
NT = BT // 128             # 32 bt tiles
ND = D // 512              # 4 D chunks

_CACHE = {}


def _build():
    nc = bacc.Bacc("TRN2", target_bir_lowering=False, debug=False,
                   num_devices=N_CORES)
    qT = nc.dram_tensor("qT", [D, BT], BF, kind="ExternalInput").ap()
    kT = nc.dram_tensor("kT", [D, BT], BF, kind="ExternalInput").ap()
    vT = nc.dram_tensor("vT", [D, BT], BF, kind="ExternalInput").ap()
    wq = nc.dram_tensor("wq", [128, KC * DL], BF, kind="ExternalInput").ap()
    wk = nc.dram_tensor("wk", [128, KC * DL], BF, kind="ExternalInput").ap()
    wv = nc.dram_tensor("wv", [128, KC * DL], BF, kind="ExternalInput").ap()
    wo = nc.dram_tensor("wo", [128, HL * D], BF, kind="ExternalInput").ap()
    maskT = nc.dram_tensor("maskT", [B, T, T], BF, kind="ExternalInput").ap()
    out = nc.dram_tensor("out", [BT, D], BF, kind="ExternalOutput").ap()

    EXP = mybir.ActivationFunctionType.Exp
    MUL = mybir.AluOpType.mult

    with tile.TileContext(nc) as tc:
        with tc.tile_pool(name="wpool", bufs=1) as wpool, \
             tc.tile_pool(name="persist", bufs=1) as ppool, \
             tc.tile_pool(name="stream", bufs=2) as spool, \
             tc.tile_pool(name="mask", bufs=18) as mpool, \
             tc.tile_pool(name="attn", bufs=17) as apool, \
             tc.tile_pool(name="small", bufs=2) as rpool, \
             tc.tile_pool(name="ostage", bufs=2) as opool, \
             tc.tile_pool(name="psmm", bufs=4, space="PSUM") as psmm, \
             tc.tile_pool(name="psacc", bufs=2, space="PSUM") as psacc, \
             tc.tile_pool(name="psden", bufs=2, space="PSUM") as psden:

            # ---- weights + constants ----
            wq_sb = wpool.tile([128, KC * DL], BF, tag="wq")
            wk_sb = wpool.tile([128, KC * DL], BF, tag="wk")
            wv_sb = wpool.tile([128, KC * DL], BF, tag="wv")
            wo_sb = wpool.tile([128, HL * D], BF, tag="wo")
            nc.sync.dma_start(wq_sb[:], wq[:])
            nc.sync.dma_start(wk_sb[:], wk[:])
            nc.sync.dma_start(wv_sb[:], wv[:])
            nc.sync.dma_start(wo_sb[:], wo[:])
            ones = wpool.tile([128, 1], BF, tag="ones")
            nc.vector.memset(ones[:], 1.0)

            # ---- persistent activations ----
            QT = [ppool.tile([128, BT], BF, tag=f"QT{h}", name=f"QT{h}") for h in range(HL)]
            KT = [ppool.tile([128, BT], BF, tag=f"KT{h}", name=f"KT{h}") for h in range(HL)]
            OT = [ppool.tile([128, BT], BF, tag=f"OT{h}", name=f"OT{h}") for h in range(HL)]
            V = [ppool.tile([128, DL], BF, tag=f"V{t}", name=f"V{t}") for t in range(NT)]

            # ---- Q, K projections (transposed outputs) ----
            for src, w_sb, dstT in ((qT, wq_sb, QT), (kT, wk_sb, KT)):
                for c in range(NBT):
                    ch = spool.tile([128, KC, 512], BF, tag="proj_in")
                    nc.sync.dma_start(
                        ch[:],
                        src[:, c * 512:(c + 1) * 512].rearrange(
                            "(kc p) j -> p kc j", p=128))
                    for m in range(HL):
                        ps = psmm.tile([128, 512], FP32, tag="mm")
                        for kc in range(KC):
                            nc.tensor.matmul(
                                ps[:],
                                w_sb[:, kc * DL + m * 128:kc * DL + (m + 1) * 128],
                                ch[:, kc, :],
                                start=kc == 0, stop=kc == KC - 1)
                        nc.vector.tensor_copy(
                            dstT[m][:, c * 512:(c + 1) * 512], ps[:])

            # ---- V projection (natural layout) ----
            for c in range(NBT):
                ch = spool.tile([128, KC, 512], BF, tag="proj_in")
                nc.sync.dma_start(
                    ch[:],
                    vT[:, c * 512:(c + 1) * 512].rearrange(
                        "(kc p) j -> p kc j", p=128))
                for sub in range(4):
                    t = c * 4 + sub
                    ps = psmm.tile([128, DL], FP32, tag="mm")
                    for kc in range(KC):
                        nc.tensor.matmul(
                            ps[:],
                            ch[:, kc, sub * 128:(sub + 1) * 128],
                            wv_sb[:, kc * DL:(kc + 1) * DL],
                            start=kc == 0, stop=kc == KC - 1)
                    nc.vector.tensor_copy(V[t][:], ps[:])

            # ---- attention ----
            for b in range(B):
                for qc in range(NQ):
                    q0 = b * T + qc * 512
                    mtiles = []
                    for kt in range(NKT):
                        mt = mpool.tile([128, 512], BF, tag="mask")
                        nc.sync.dma_start(
                            mt[:],
                            maskT[b, kt * 128:(kt + 1) * 128,
                                  qc * 512:(qc + 1) * 512])
                        mtiles.append(mt)
                    for h in range(HL):
                        atiles = []
                        for kt in range(NKT):
                            ps_s = psmm.tile([128, 512], FP32, tag="mm")
                            nc.tensor.matmul(
                                ps_s[:],
                                KT[h][:, b * T + kt * 128:b * T + (kt + 1) * 128],
                                QT[h][:, q0:q0 + 512],
                                start=True, stop=True)
                            at = apool.tile([128, 512], BF, tag="attn")
                            nc.scalar.activation(at[:], ps_s[:], EXP)
                            nc.vector.tensor_tensor(
                                at[:], at[:], mtiles[kt][:], op=MUL)
                            atiles.append(at)
                        ps_d = psden.tile([1, 512], FP32, tag="den")
                        for kt in range(NKT):
                            nc.tensor.matmul(
                                ps_d[:], ones[:], atiles[kt][:],
                                start=kt == 0, stop=kt == NKT - 1)
                        rec = rpool.tile([1, 512], FP32, tag="rec")
                        nc.vector.reciprocal(rec[:], ps_d[:])
                        rbc = rpool.tile([128, 512], FP32, tag="rbc")
                        nc.gpsimd.partition_broadcast(rbc[:], rec[:])
                        ps_o = psacc.tile([128, 512], FP32, tag="acc")
                        for kt in range(NKT):
                            nc.tensor.matmul(
                                ps_o[:],
                                V[b * NKT + kt][:, h * 128:(h + 1) * 128],
                                atiles[kt][:],
                                start=kt == 0, stop=kt == NKT - 1)
                        nc.vector.scalar_tensor_tensor(
                            OT[h][:, q0:q0 + 512], ps_o[:], 1.0, rbc[:],
                            op0=MUL, op1=MUL)

            # ---- output projection ----
            for t in range(NT):
                stage = opool.tile([128, D], BF, tag="ostage")
                for dc in range(ND):
                    ps = psmm.tile([128, 512], FP32, tag="mm")
                    for h in range(HL):
                        nc.tensor.matmul(
                            ps[:],
                            OT[h][:, t * 128:(t + 1) * 128],
                            wo_sb[:, h * D + dc * 512:h * D + (dc + 1) * 512],
                            start=h == 0, stop=h == HL - 1)
                    nc.scalar.copy(stage[:, dc * 512:(dc + 1) * 512], ps[:])
                nc.sync.dma_start(out[t * 128:(t + 1) * 128, :], stage[:])

    nc.compile()
    return nc


def get_nc():
    if "nc" not in _CACHE:
        _CACHE["nc"] = _build()
    return _CACHE["nc"]


def make_in_maps(q, k, v, Wq, Wk, Wv, Wo, attn_mask, key_padding_mask):
    scale = np.float32(1.0 / np.sqrt(np.float32(DH)))
    qT = np.ascontiguousarray(q.reshape(BT, D).T.astype(BF16))
    kT = np.ascontiguousarray(k.reshape(BT, D).T.astype(BF16))
    vT = np.ascontiguousarray(v.reshape(BT, D).T.astype(BF16))
    # multiplicative transposed mask [B, TK, TQ]
    m = ~(key_padding_mask[:, :, None] | attn_mask.T[None, :, :])
    maskT = np.ascontiguousarray(m.astype(BF16))

    def prep_w(wT):  # [D, DL] -> [128, KC*DL]
        return np.ascontiguousarray(
            wT.reshape(KC, 128, DL).transpose(1, 0, 2).reshape(128, KC * DL)
            .astype(BF16))

    in_maps = []
    for i in range(N_CORES):
        rows = slice(i * DL, (i + 1) * DL)
        wq_i = prep_w(Wq[rows, :].T * scale)
        wk_i = prep_w(Wk[rows, :].T)
        wv_i = prep_w(Wv[rows, :].T)
        woT = Wo[:, rows].T  # [DL, D]
        wo_i = np.ascontiguousarray(
            woT.reshape(HL, 128, D).transpose(1, 0, 2).reshape(128, HL * D)
            .astype(BF16))
        in_maps.append({
            "qT": qT, "kT": kT, "vT": vT,
            "wq": wq_i, "wk": wk_i, "wv": wv_i, "wo": wo_i,
            "maskT": maskT,
        })
    return in_maps


def postprocess(results):
    acc = np.zeros((BT, D), np.float32)
    for r in results:
        acc += r["out"].astype(np.float32)
    return acc.reshape(B, T, D)


def kernel(**inputs):
    inputs = {k: np.asarray(v) for k, v in inputs.items()}
    nc = get_nc()
    in_maps = make_in_maps(**inputs)
    res = bass_utils.run_bass_kernel_spmd(
        nc, in_maps, core_ids=list(range(N_CORES)))
    return postprocess(res.results)


# revision 4
# speedup vs baseline: 1.1143x; 1.1143x over previous
"""Multi-head attention (B=2, T=2048, D=2048, H=16) on 8 TRN2 NeuronCores.

Tensor-parallel over heads: each core computes 2 heads (dl=256 of D) of the
Q/K/V projections, its heads' attention, and a partial output projection
(columns of Wo). Host sums the 8 partial outputs (the "all-reduce").

Per-core dataflow (bf16 compute, f32 PSUM accumulation):
  QT_h = (Wq_h/sqrt(dh)) @ q^T        [dh=128, BT=4096]  (transposed layout)
  KT_h = Wk_h @ k^T                   [dh, BT]
  V    = v @ Wv_i^T                   [BT, 256]           (natural layout)
  scoresT = KT_h-chunk.T @ QT_h       [k-tile 128, q 512] per (b, h)
  attnT = exp(scoresT) * maskT        (mask multiplicative {0,1}; no max
                                       subtraction needed: |scores| < ~8)
  denom = ones.T @ attnT              (PE column-sum over k, f32 PSUM)
  O^T_h = (V_h-chunk.T @ attnT) * (1/denom)
  partial = O^T.T @ Wo_i^T            [BT, D] -> host sum over cores

Work is emitted batch-by-batch (proj b, attention b, out-proj b) with
fine-grained tiles so the Tile scheduler overlaps phases.
"""
import sys

if "/opt/trn_rl_repo" not in sys.path:
    sys.path.insert(0, "/opt/trn_rl_repo")

import numpy as np
import ml_dtypes

import concourse.bacc as bacc
import concourse.tile as tile
import concourse.mybir as mybir
from concourse import bass_utils

BF16 = ml_dtypes.bfloat16
FP32 = mybir.dt.float32
BF = mybir.dt.bfloat16

B, T, D, H = 2, 2048, 2048, 16
DH = 128
N_CORES = 8
HL = H // N_CORES          # heads per core = 2
DL = HL * DH               # local out dim = 256
BT = B * T                 # 4096
KC = D // 128              # 16 contraction chunks
NQ = T // 512              # 4 q chunks per batch
NKT = T // 128             # 16 k tiles per batch
ND = D // 512              # 4 D chunks

_CACHE = {}


def _build():
    nc = bacc.Bacc("TRN2", target_bir_lowering=False, debug=False,
                   num_devices=N_CORES)
    qT = nc.dram_tensor("qT", [D, BT], BF, kind="ExternalInput").ap()
    kT = nc.dram_tensor("kT", [D, BT], BF, kind="ExternalInput").ap()
    vT = nc.dram_tensor("vT", [D, BT], BF, kind="ExternalInput").ap()
    wq = nc.dram_tensor("wq", [128, KC * DL], BF, kind="ExternalInput").ap()
    wk = nc.dram_tensor("wk", [128, KC * DL], BF, kind="ExternalInput").ap()
    wv = nc.dram_tensor("wv", [128, KC * DL], BF, kind="ExternalInput").ap()
    wo = nc.dram_tensor("wo", [128, HL * D], BF, kind="ExternalInput").ap()
    maskT = nc.dram_tensor("maskT", [B, T, T], BF, kind="ExternalInput").ap()
    out = nc.dram_tensor("out", [BT, D], BF, kind="ExternalOutput").ap()

    EXP = mybir.ActivationFunctionType.Exp
    MUL = mybir.AluOpType.mult

    with tile.TileContext(nc) as tc:
        with tc.tile_pool(name="wpool", bufs=1) as wpool, \
             tc.tile_pool(name="persist", bufs=1) as ppool, \
             tc.tile_pool(name="stream", bufs=6) as spool, \
             tc.tile_pool(name="mask", bufs=17) as mpool, \
             tc.tile_pool(name="attn", bufs=12) as apool, \
             tc.tile_pool(name="small", bufs=2) as rpool, \
             tc.tile_pool(name="ostage", bufs=2) as opool, \
             tc.tile_pool(name="psbig", bufs=2, space="PSUM") as psbig, \
             tc.tile_pool(name="psacc", bufs=2, space="PSUM") as psacc, \
             tc.tile_pool(name="psden", bufs=2, space="PSUM") as psden:

            # ---- weights + constants ----
            wq_sb = wpool.tile([128, KC * DL], BF, tag="wq")
            wk_sb = wpool.tile([128, KC * DL], BF, tag="wk")
            wv_sb = wpool.tile([128, KC * DL], BF, tag="wv")
            wo_sb = wpool.tile([128, HL * D], BF, tag="wo")
            nc.sync.dma_start(wq_sb[:], wq[:])
            nc.sync.dma_start(wk_sb[:], wk[:])
            nc.sync.dma_start(wv_sb[:], wv[:])
            nc.sync.dma_start(wo_sb[:], wo[:])
            ones = wpool.tile([128, 1], BF, tag="ones")
            nc.vector.memset(ones[:], 1.0)

            # ---- persistent activations (fine-grained for overlap) ----
            # QT[h][c]: [128, 512] transposed Q, c = global bt chunk (8)
            QT = [[ppool.tile([128, 512], BF, tag=f"QT{h}_{c}", name=f"QT{h}_{c}")
                   for c in range(2 * NQ)] for h in range(HL)]
            # KT[h][b]: [128, 2048]
            KT = [[ppool.tile([128, T], BF, tag=f"KT{h}_{b}", name=f"KT{h}_{b}")
                   for b in range(B)] for h in range(HL)]
            # OT[h][b*NQ+qc]: [128, 512]
            OT = [[ppool.tile([128, 512], BF, tag=f"OT{h}_{c}", name=f"OT{h}_{c}")
                   for c in range(2 * NQ)] for h in range(HL)]
            # V[t]: [128, DL] natural, t = global bt tile (32)
            V = [ppool.tile([128, DL], BF, tag=f"V{t}", name=f"V{t}")
                 for t in range(BT // 128)]

            def emit_proj(b):
                for cc in range(NQ):          # 4 chunks of 512 per batch
                    c = b * NQ + cc
                    # two half-chunks (kc 0-7 / 8-15) per tensor, shared slots
                    halves = {}
                    for nm, srct in (("q", qT), ("k", kT), ("v", vT)):
                        for hf in range(2):
                            ch = spool.tile([128, KC // 2, 512], BF,
                                            tag="pin", name=f"pin_{nm}{hf}")
                            nc.sync.dma_start(
                                ch[:],
                                srct[hf * (D // 2):(hf + 1) * (D // 2),
                                     c * 512:(c + 1) * 512].rearrange(
                                    "(kc p) j -> p kc j", p=128))
                            halves[nm, hf] = ch
                    def chx(nm, kc):
                        return halves[nm, kc // (KC // 2)][:, kc % (KC // 2), :]
                    for m in range(HL):
                        ps = psbig.tile([128, 1024], FP32, tag="mm")
                        for kc in range(KC):
                            nc.tensor.matmul(
                                ps[:, :512],
                                wq_sb[:, kc * DL + m * 128:kc * DL + (m + 1) * 128],
                                chx("q", kc),
                                start=kc == 0, stop=kc == KC - 1)
                        nc.vector.tensor_copy(QT[m][c][:], ps[:, :512])
                        ps2 = psbig.tile([128, 1024], FP32, tag="mm")
                        for kc in range(KC):
                            nc.tensor.matmul(
                                ps2[:, :512],
                                wk_sb[:, kc * DL + m * 128:kc * DL + (m + 1) * 128],
                                chx("k", kc),
                                start=kc == 0, stop=kc == KC - 1)
                        nc.vector.tensor_copy(
                            KT[m][b][:, cc * 512:(cc + 1) * 512], ps2[:, :512])
                    for sub in range(4):
                        t = c * 4 + sub
                        psv = psacc.tile([128, 512], FP32, tag="acc")
                        for kc in range(KC):
                            nc.tensor.matmul(
                                psv[:, :DL],
                                chx("v", kc)[:, sub * 128:(sub + 1) * 128],
                                wv_sb[:, kc * DL:(kc + 1) * DL],
                                start=kc == 0, stop=kc == KC - 1)
                        nc.vector.tensor_copy(V[t][:], psv[:, :DL])

            def emit_attention(b):
                for qc in range(NQ):
                    c = b * NQ + qc
                    mtiles = []
                    for kt in range(NKT):
                        mt = mpool.tile([128, 512], BF, tag="mask")
                        nc.sync.dma_start(
                            mt[:],
                            maskT[b, kt * 128:(kt + 1) * 128,
                                  qc * 512:(qc + 1) * 512])
                        mtiles.append(mt)
                    for h in range(HL):
                        atiles = []
                        for kp in range(NKT // 2):   # pairs of k tiles
                            ps_s = psbig.tile([128, 1024], FP32, tag="mm")
                            for i in range(2):
                                kt = kp * 2 + i
                                nc.tensor.matmul(
                                    ps_s[:, i * 512:(i + 1) * 512],
                                    KT[h][b][:, kt * 128:(kt + 1) * 128],
                                    QT[h][c][:],
                                    start=True, stop=True)
                            ap2 = apool.tile([128, 1024], BF, tag="attn")
                            nc.scalar.activation(ap2[:], ps_s[:], EXP)
                            for i in range(2):
                                kt = kp * 2 + i
                                nc.vector.tensor_tensor(
                                    ap2[:, i * 512:(i + 1) * 512],
                                    ap2[:, i * 512:(i + 1) * 512],
                                    mtiles[kt][:], op=MUL)
                            atiles.append(ap2)
                        ps_d = psden.tile([1, 512], FP32, tag="den")
                        for kp in range(NKT // 2):
                            for i in range(2):
                                nc.tensor.matmul(
                                    ps_d[:], ones[:],
                                    atiles[kp][:, i * 512:(i + 1) * 512],
                                    start=(kp == 0 and i == 0),
                                    stop=(kp == NKT // 2 - 1 and i == 1))
                        rec = rpool.tile([1, 512], FP32, tag="rec")
                        nc.vector.reciprocal_approx_fast(rec[:], ps_d[:])
                        rbc = rpool.tile([128, 512], FP32, tag="rbc")
                        nc.gpsimd.partition_broadcast(rbc[:], rec[:])
                        ps_o = psacc.tile([128, 512], FP32, tag="acc")
                        for kp in range(NKT // 2):
                            for i in range(2):
                                kt = kp * 2 + i
                                nc.tensor.matmul(
                                    ps_o[:],
                                    V[b * NKT + kt][:, h * 128:(h + 1) * 128],
                                    atiles[kp][:, i * 512:(i + 1) * 512],
                                    start=(kp == 0 and i == 0),
                                    stop=(kp == NKT // 2 - 1 and i == 1))
                        nc.vector.scalar_tensor_tensor(
                            OT[h][c][:], ps_o[:], 1.0, rbc[:],
                            op0=MUL, op1=MUL)

            def emit_outproj(b):
                for qc in range(NQ):
                    c = b * NQ + qc
                    for sub in range(4):
                        t = c * 4 + sub
                        stage = opool.tile([128, D], BF, tag="ostage")
                        for dp in range(2):      # two 1024-wide halves of D
                            ps = psbig.tile([128, 1024], FP32, tag="mm")
                            for i in range(2):
                                dc = dp * 2 + i
                                for h in range(HL):
                                    nc.tensor.matmul(
                                        ps[:, i * 512:(i + 1) * 512],
                                        OT[h][c][:, sub * 128:(sub + 1) * 128],
                                        wo_sb[:, h * D + dc * 512:
                                              h * D + (dc + 1) * 512],
                                        start=h == 0, stop=h == HL - 1)
                            if dp == 0:
                                nc.scalar.copy(stage[:, :1024], ps[:])
                            else:
                                nc.vector.tensor_copy(stage[:, 1024:], ps[:])
                        nc.sync.dma_start(out[t * 128:(t + 1) * 128, :], stage[:])

            for b in range(B):
                emit_proj(b)
                emit_attention(b)
                emit_outproj(b)

    nc.compile()
    return nc


def get_nc():
    if "nc" not in _CACHE:
        _CACHE["nc"] = _build()
    return _CACHE["nc"]


def make_in_maps(q, k, v, Wq, Wk, Wv, Wo, attn_mask, key_padding_mask):
    scale = np.float32(1.0 / np.sqrt(np.float32(DH)))
    qT = np.ascontiguousarray(q.reshape(BT, D).T.astype(BF16))
    kT = np.ascontiguousarray(k.reshape(BT, D).T.astype(BF16))
    vT = np.ascontiguousarray(v.reshape(BT, D).T.astype(BF16))
    # multiplicative transposed mask [B, TK, TQ]
    m = ~(key_padding_mask[:, :, None] | attn_mask.T[None, :, :])
    maskT = np.ascontiguousarray(m.astype(BF16))

    def prep_w(wT):  # [D, DL] -> [128, KC*DL]
        return np.ascontiguousarray(
            wT.reshape(KC, 128, DL).transpose(1, 0, 2).reshape(128, KC * DL)
            .astype(BF16))

    in_maps = []
    for i in range(N_CORES):
        rows = slice(i * DL, (i + 1) * DL)
        wq_i = prep_w(Wq[rows, :].T * scale)
        wk_i = prep_w(Wk[rows, :].T)
        wv_i = prep_w(Wv[rows, :].T)
        woT = Wo[:, rows].T  # [DL, D]
        wo_i = np.ascontiguousarray(
            woT.reshape(HL, 128, D).transpose(1, 0, 2).reshape(128, HL * D)
            .astype(BF16))
        in_maps.append({
            "qT": qT, "kT": kT, "vT": vT,
            "wq": wq_i, "wk": wk_i, "wv": wv_i, "wo": wo_i,
            "maskT": maskT,
        })
    return in_maps


def postprocess(results):
    acc = np.zeros((BT, D), np.float32)
    for r in results:
        acc += r["out"].astype(np.float32)
    return acc.reshape(B, T, D)


def kernel(**inputs):
    inputs = {k: np.asarray(v) for k, v in inputs.items()}
    nc = get_nc()
    in_maps = make_in_maps(**inputs)
    res = bass_utils.run_bass_kernel_spmd(
        nc, in_maps, core_ids=list(range(N_CORES)))
    return postprocess(res.results)
